# revision 20
# baseline (speedup 1.0000x reference)
"""Trainium2 Bass kernel for nn_ConvLayer_13967233646751 (gnn_message_passing).

Reference computation (per batch b, point p, neighbor s):
  - build local frame R from normal + azimuth (mean of rel coords over s=1..31)
  - x = [R@rel, feats, R@other_normal, R@azi_u - R@other_dir]   (73 ch)
  - h = relu(W2 @ relu(W1 @ x + b1) + b2); pooled = max over s
  - out = concat([azi_u, pooled])  -> (B, 131, P)

Sharding: data-parallel over batch B=16 across 8 cores (2 batches/core).

v4 device design (per core), pipelined per (b, qt) unit:
  - bulk loads (geo, consts, feats) on gpsimd SWDGE (async posting, descriptors
    spread across all 16 SDMA engines); ordered geo0, consts, geo1, feats so
    the R phase and first GEMMs unblock early.
  - R phase on DVE, batched over both batches (halves small-op overhead).
  - rotation per (b, half=2qt): 16 TT ops [128,4,3,S] on gpsimd -> alq16 fp16.
  - scatter alq16 -> xt rows 64:73 via p-major DRAM scratch on the otherwise
    idle sync HWDGE ring (write contiguous 576B/partition, read 64B strided).
  - GEMM per (b,qt): 8 chunks of 1024 cols; tensor stream software-pipelined
    (G1(k+1) issued before G2(k)).
  - h1 evac: ACT relu+bias -> fp16.
  - pool: DVE reduce_max from PSUM; last GP_NPOOL chunks per quarter go
    ACT-evac + gpsimd reduce; per-quarter raw-max tiles avoid false deps;
    relu+b2 applied per-quarter (max commutes with monotone relu).
"""

import numpy as np
from contextlib import ExitStack

import concourse.bass as bass
import concourse.tile as tile
from concourse import bacc
from concourse import mybir
from concourse.bass_utils import run_bass_kernel_spmd

F32 = mybir.dt.float32
F16 = mybir.dt.float16
AX = mybir.AxisListType
OP = mybir.AluOpType
AF = mybir.ActivationFunctionType

EPS = 1e-8
B, C, P, S = 16, 76, 1024, 32
NCORES = 8
BL = B // NCORES          # batches per core
NQ = P // 128             # 8 q-groups per batch
QTR = P // 256            # 4 quarters per batch
NCH = 8                   # 1024-col chunks per quarter

# geo channel c = f*3+i -> w1 column (f: 0=nrm, 1=rel, 2=dir)
GEO_W1_COLS = [67, 68, 69, 0, 1, 2, 70, 71, 72]
# per qt: tail chunks pooled via ACT-evac + DVE fp16 reduce (off the PSUM path)
GP_NPOOL = {0: 1, 1: 1, 2: 1, 3: 1}
# rotation engine per (b, half): 'g' = gpsimd, 'v' = vector.  b=0 halves run on
# the DVE, which is idle before the GEMM pipeline fills; b=1 on gpsimd.
ROT_ENG = {(0, 0): 'v', (0, 1): 'v', (1, 0): 'g', (1, 1): 'g'}


def build_program():
    nc = bacc.Bacc()

    feats_d = nc.dram_tensor("feats", [BL, QTR, 64, 2, 128, S], F16, kind="ExternalInput")
    geo_d = nc.dram_tensor("geo", [BL, 128, 3, 3, NQ, S], F16, kind="ExternalInput")
    norm_d = nc.dram_tensor("normp", [BL, 128, 3, NQ], F32, kind="ExternalInput")
    w1c_d = nc.dram_tensor("w1c", [73, 128], F16, kind="ExternalInput")
    w2T_d = nc.dram_tensor("w2T", [128, 128], F16, kind="ExternalInput")
    b1_d = nc.dram_tensor("b1c", [128, 1], F32, kind="ExternalInput")
    b2_d = nc.dram_tensor("b2c", [128, 1], F32, kind="ExternalInput")
    outp_d = nc.dram_tensor("outp", [BL, 128, P], F32, kind="ExternalOutput")
    outa_d = nc.dram_tensor("outa", [BL, 128, 3, NQ], F32, kind="ExternalOutput")
    # p-major geo scratch: contiguous DRAM writes (no RMW), 64B strided reads
    scrg_d = nc.dram_tensor("scrg", [BL, QTR, 2, 128, 9, S], F16, kind="Internal")

    with tile.TileContext(nc) as tc, ExitStack() as ctx:
        cpool = ctx.enter_context(tc.tile_pool(name="const", bufs=1))
        geo_pool = ctx.enter_context(tc.tile_pool(name="geo", bufs=2))
        rpool = ctx.enter_context(tc.tile_pool(name="rphase", bufs=1))
        alq_pool = ctx.enter_context(tc.tile_pool(name="aligned", bufs=3))
        tmp_pool = ctx.enter_context(tc.tile_pool(name="rtmp", bufs=3))
        xt_pool = ctx.enter_context(tc.tile_pool(name="xt", bufs=8))
        h1_pool = ctx.enter_context(tc.tile_pool(name="h1", bufs=4))
        pb_pool = ctx.enter_context(tc.tile_pool(name="pooled", bufs=3))
        po_pool = ctx.enter_context(tc.tile_pool(name="poolout", bufs=2))
        ps1_pool = ctx.enter_context(tc.tile_pool(name="ps1", bufs=2, space="PSUM"))
        ps2_pool = ctx.enter_context(tc.tile_pool(name="ps2", bufs=2, space="PSUM"))

        # ---- bulk loads on gpsimd SWDGE (async posting, spread engines).
        #      Order matters: geo0, consts, geo1, then feats. ----
        geo_pts = {}
        geo_pts[0] = geo_pool.tile([128, 3, 3, NQ, S], F16, tag="geo_pt", name="geo0")
        nc.gpsimd.dma_start(out=geo_pts[0][:], in_=geo_d[0])

        norm_pt = cpool.tile([128, BL, 3, NQ], F32)
        nc.gpsimd.dma_start(out=norm_pt[:], in_=norm_d[:, :, :, :].rearrange("b p x q -> p b x q"))
        w1c = cpool.tile([73, 128], F16)
        nc.gpsimd.dma_start(out=w1c[:], in_=w1c_d[:, :])
        w2T = cpool.tile([128, 128], F16)
        nc.gpsimd.dma_start(out=w2T[:], in_=w2T_d[:, :])
        b1t = cpool.tile([128, 1], F32)
        nc.gpsimd.dma_start(out=b1t[:], in_=b1_d[:, :])
        b2t = cpool.tile([128, 1], F32)
        nc.gpsimd.dma_start(out=b2t[:], in_=b2_d[:, :])

        geo_pts[1] = geo_pool.tile([128, 3, 3, NQ, S], F16, tag="geo_pt", name="geo1")
        nc.gpsimd.dma_start(out=geo_pts[1][:], in_=geo_d[1])

        xts = {}
        for b in range(BL):
            for qt in range(QTR):
                xt = xt_pool.tile([73, 2 * 128 * S], F16, tag="xt", name=f"xt_{b}_{qt}")
                xts[(b, qt)] = xt
                nc.gpsimd.dma_start(
                    out=xt[0:64].rearrange("c (h p s) -> c h p s", h=2, p=128, s=S),
                    in_=feats_d[b, qt],
                )

        # ---- R phase, batched over both batches.
        # The frame is scale-invariant in azi, so the critical chain to Rt16
        # skips normalizing azi: x_u = unit(azi - (azi.n)n); a_u / svec are
        # recovered off-chain via inva = 1/(|azi|+eps):
        #   a_u = azi*inva,  svec = (R @ azi)*inva. ----
        def uops(v, u, sq, ss, nrm, inv):
            """u = v/(||v||+eps) along xyz; v,u,sq: [128,BL,3,NQ]; ss,nrm,inv: [128,BL,NQ]."""
            nc.vector.tensor_tensor(out=sq[:], in0=v, in1=v, op=OP.mult)
            nc.vector.reduce_sum(out=ss[:], in_=sq[:].transpose([0, 1, 3, 2]), axis=AX.X)
            nc.scalar.sqrt(nrm[:], ss[:])
            nc.vector.tensor_scalar_add(out=nrm[:], in0=nrm[:], scalar1=EPS)
            nc.vector.reciprocal(inv[:], nrm[:])
            inv_b = inv[:].unsqueeze(2).broadcast_to([128, BL, 3, NQ])
            nc.vector.tensor_tensor(out=u, in0=v, in1=inv_b, op=OP.mult)

        Rt = rpool.tile([128, BL, 3, 3, NQ], F32, tag="Rt")
        azi = rpool.tile([128, BL, 3, NQ], F32, tag="azi")
        sq = rpool.tile([128, BL, 3, NQ], F32, tag="sq")
        ss = rpool.tile([128, BL, NQ], F32, tag="ss")
        nrm = rpool.tile([128, BL, NQ], F32, tag="nrm")
        inv = rpool.tile([128, BL, NQ], F32, tag="inv")
        a_u = rpool.tile([128, BL, 3, NQ], F32, tag="a_u")
        dotp = rpool.tile([128, BL, NQ], F32, tag="dotp")
        xraw = rpool.tile([128, BL, 3, NQ], F32, tag="xraw")
        razi = rpool.tile([128, BL, 3, NQ], F32, tag="razi")
        tmp3 = rpool.tile([128, BL, 3, NQ], F32, tag="tmp3")
        Rt16 = rpool.tile([128, BL, 3, 3, NQ], F16, tag="Rt16")
        sv16 = rpool.tile([128, BL, 3, NQ], F16, tag="sv16")

        x_u = Rt[:, :, 0]
        yax = Rt[:, :, 1]
        n_u = Rt[:, :, 2]

        # unit(normal) -- independent of the geo DMAs, starts immediately
        uops(norm_pt[:], n_u, sq, ss, nrm, inv)

        # azi = (sum_s rel - rel[s=0]) / 31     (mean over s=1..31)
        for b in range(BL):
            nc.vector.reduce_sum(out=azi[:, b], in_=geo_pts[b][:, 1], axis=AX.X)
            nc.vector.tensor_tensor(out=azi[:, b], in0=azi[:, b],
                                    in1=geo_pts[b][:, 1, :, :, 0], op=OP.subtract)
        nc.vector.tensor_scalar_mul(out=azi[:], in0=azi[:], scalar1=1.0 / 31.0)

        # dotp = sum_xyz azi*n_u ; xraw = azi - dotp*n_u ; x_u = unit(xraw)
        nc.vector.tensor_tensor(out=tmp3[:], in0=azi[:], in1=n_u, op=OP.mult)
        nc.vector.reduce_sum(out=dotp[:], in_=tmp3[:].transpose([0, 1, 3, 2]), axis=AX.X)
        dot_b = dotp[:].unsqueeze(2).broadcast_to([128, BL, 3, NQ])
        nc.vector.tensor_tensor(out=xraw[:], in0=dot_b, in1=n_u, op=OP.mult)
        nc.vector.tensor_tensor(out=xraw[:], in0=azi[:], in1=xraw[:], op=OP.subtract)
        uops(xraw[:], x_u, sq, ss, nrm, inv)

        # yax = cross(n_u, x_u)
        for x_ in range(3):
            i1, i2 = (x_ + 1) % 3, (x_ + 2) % 3
            nc.vector.tensor_tensor(out=yax[:, :, x_], in0=n_u[:, :, i1], in1=x_u[:, :, i2], op=OP.mult)
            nc.vector.tensor_tensor(out=tmp3[:, :, x_], in0=n_u[:, :, i2], in1=x_u[:, :, i1], op=OP.mult)
        nc.vector.tensor_tensor(out=yax, in0=yax, in1=tmp3[:], op=OP.subtract)
        nc.vector.tensor_copy(out=Rt16[:], in_=Rt[:])

        # ---- off-chain: inva, a_u, svec = (R@azi)*inva ----
        nc.vector.tensor_tensor(out=sq[:], in0=azi[:], in1=azi[:], op=OP.mult)
        nc.vector.reduce_sum(out=ss[:], in_=sq[:].transpose([0, 1, 3, 2]), axis=AX.X)
        nc.scalar.sqrt(nrm[:], ss[:])
        nc.vector.tensor_scalar_add(out=nrm[:], in0=nrm[:], scalar1=EPS)
        nc.vector.reciprocal(inv[:], nrm[:])
        inv_b = inv[:].unsqueeze(2).broadcast_to([128, BL, 3, NQ])
        nc.vector.tensor_tensor(out=a_u[:], in0=azi[:], in1=inv_b, op=OP.mult)
        nc.vector.tensor_tensor(out=tmp3[:], in0=x_u, in1=azi[:], op=OP.mult)
        nc.vector.reduce_sum(out=razi[:, :, 0], in_=tmp3[:].transpose([0, 1, 3, 2]), axis=AX.X)
        nc.vector.tensor_tensor(out=tmp3[:], in0=yax, in1=azi[:], op=OP.mult)
        nc.vector.reduce_sum(out=razi[:, :, 1], in_=tmp3[:].transpose([0, 1, 3, 2]), axis=AX.X)
        nc.vector.tensor_copy(out=razi[:, :, 2], in_=dotp[:])
        nc.vector.tensor_tensor(out=sv16[:], in0=razi[:], in1=inv_b, op=OP.mult)
        for b in range(BL):
            nc.gpsimd.dma_start(out=outa_d[b], in_=a_u[:, b])

        # ---- rotation / scatter / GEMM pipeline ----
        def rot_half(b, half):
            """alq[p, ql, f, i, s] = sum_j Rt[p,b,i,j,q] * geo[p,f,j,q,s]; f=2: svec - ."""
            geo_pt = geo_pts[b]
            qs = slice(4 * half, 4 * half + 4)
            eng = nc.gpsimd if ROT_ENG[(b, half)] == 'g' else nc.vector
            alq16 = alq_pool.tile([128, 4, 3, 3, S], F16, tag="alq16")
            for f in range(3):
                out_f = alq16[:, :, f]                       # [128, 4, 3, S]
                for j in range(3):
                    g_v = geo_pt[:, f, j, qs, :].unsqueeze(2).broadcast_to([128, 4, 3, S])
                    r_v = Rt16[:, b, :, j, qs].transpose([0, 2, 1]) \
                        .unsqueeze(3).broadcast_to([128, 4, 3, S])
                    if j == 0:
                        eng.tensor_tensor(out=out_f, in0=g_v, in1=r_v, op=OP.mult)
                    else:
                        t = tmp_pool.tile([128, 4, 3, S], F16, tag="rtmp")
                        eng.tensor_tensor(out=t[:], in0=g_v, in1=r_v, op=OP.mult)
                        eng.tensor_tensor(out=out_f, in0=out_f, in1=t[:], op=OP.add)
            sv_b = sv16[:, b, :, qs].transpose([0, 2, 1]).unsqueeze(3).broadcast_to([128, 4, 3, S])
            eng.tensor_tensor(out=alq16[:, :, 2], in0=sv_b, in1=alq16[:, :, 2], op=OP.subtract)
            return alq16

        def scatter(b, qt, alq16):
            # xt rows 64:73 (cols (h,p,s)) via p-major DRAM scratch; sync HWDGE
            # blocks per transfer but the ring is otherwise idle.
            xt = xts[(b, qt)]
            ql0 = 2 * (qt % 2)
            nc.sync.dma_start(
                out=scrg_d[b, qt].rearrange("h p c s -> p h (c s)"),
                in_=alq16[:, ql0:ql0 + 2].rearrange("p q f i s -> p q (f i s)"),
            )
            nc.sync.dma_start(
                out=xt[64:73].rearrange("c (h p s) -> c (h p) s", h=2, p=128, s=S),
                in_=scrg_d[b, qt].rearrange("h p c s -> c (h p) s"),
            )

        def gemm_unit(b, qt):
            xt = xts[(b, qt)]
            pooled_o = pooled[b]
            pooled_q = pb_pool.tile([128, 256], F32, tag="pooled_raw")
            npool = GP_NPOOL[qt]
            h1_of = {}
            # 2-chunk weight groups: [G1(2g) G1(2g+1)] [G2(2g-2) G2(2g-1)] so
            # the PE runs 4 same-weight matmuls per LDWEIGHTS swap and each
            # evac has two full matmul-pairs of slack before its G2.
            for g in range(NCH // 2 + 1):
                if g < NCH // 2:
                    for k in (2 * g, 2 * g + 1):
                        h1ps = ps1_pool.tile([128, 1024], F32, tag="h1ps")
                        for half in range(2):
                            off = 1024 * k + 512 * half
                            nc.tensor.matmul(out=h1ps[:, 512 * half:512 * half + 512],
                                             lhsT=w1c[:], rhs=xt[:, off:off + 512],
                                             start=True, stop=True)
                        h1sb = h1_pool.tile([128, 1024], F16, tag="h1sb")
                        h1_of[k] = h1sb
                        nc.scalar.activation(h1sb[:], h1ps[:], AF.Relu, bias=b1t[:, 0:1])
                if g > 0:
                    for kk in (2 * g - 2, 2 * g - 1):
                        h2ps = ps2_pool.tile([128, 1024], F32, tag="h2ps")
                        h1sb = h1_of.pop(kk)
                        nc.tensor.matmul(out=h2ps[:, 0:512], lhsT=w2T[:],
                                         rhs=h1sb[:, 0:512], start=True, stop=True)
                        nc.tensor.matmul(out=h2ps[:, 512:1024], lhsT=w2T[:],
                                         rhs=h1sb[:, 512:1024], start=True, stop=True)
                        po = 256 * qt + kk * 32
                        if kk >= NCH - npool:
                            # ACT evac (relu+bias, fp16) + DVE fp16 reduce
                            h2sb = h1_pool.tile([128, 1024], F16, tag="h2sb")
                            nc.scalar.activation(h2sb[:], h2ps[:], AF.Relu, bias=b2t[:, 0:1])
                            nc.vector.reduce_max(
                                out=pooled_o[:, po:po + 32],
                                in_=h2sb[:].rearrange("m (p s) -> m p s", s=S),
                                axis=AX.X)
                        else:
                            nc.vector.reduce_max(
                                out=pooled_q[:, kk * 32:kk * 32 + 32],
                                in_=h2ps[:].rearrange("m (p s) -> m p s", s=S),
                                axis=AX.X)
            # tail: relu(pooled + b2) on the DVE-path positions of this quarter
            seg = 32 * (NCH - npool)
            nc.scalar.activation(pooled_o[:, 256 * qt:256 * qt + seg],
                                 pooled_q[:, 0:seg], AF.Relu, bias=b2t[:, 0:1])
            if qt == QTR - 1:
                nc.gpsimd.dma_start(out=outp_d[b], in_=pooled_o[:])

        units = [(b, qt) for b in range(BL) for qt in range(QTR)]
        pooled = {}
        alq_cur = {}
        for u, (b, qt) in enumerate(units):
            if qt == 0:
                pooled[b] = po_pool.tile([128, P], F32, tag="pooled_out", name=f"po_{b}")
            if qt % 2 == 0:
                alq_cur[b] = rot_half(b, qt // 2)
            scatter(b, qt, alq_cur[b])
            if u > 0:
                gemm_unit(*units[u - 1])
        gemm_unit(*units[-1])

    nc.finalize()
    return nc


_CACHE = {}


def _get_program():
    if "nc" not in _CACHE:
        _CACHE["nc"] = build_program()
    return _CACHE["nc"]


def make_in_maps(input, normal, w1, b1, w2, b2):
    input = np.asarray(input, dtype=np.float32)
    normal = np.asarray(normal, dtype=np.float32)
    w1 = np.asarray(w1, dtype=np.float32)
    b1 = np.asarray(b1, dtype=np.float32)
    w2 = np.asarray(w2, dtype=np.float32)
    b2 = np.asarray(b2, dtype=np.float32)

    w1fT = w1[:, 3:67].T.astype(np.float16)                  # (64,128)
    w1gT = w1[:, GEO_W1_COLS].T.astype(np.float16)           # (9,128)
    w1c = np.ascontiguousarray(np.concatenate([w1fT, w1gT], axis=0))  # (73,128)
    w2T = np.ascontiguousarray(w2.T.astype(np.float16))      # (128,128)
    b1c = np.ascontiguousarray(b1.reshape(128, 1))
    b2c = np.ascontiguousarray(b2.reshape(128, 1))

    in_maps = []
    for core in range(NCORES):
        b0 = core * BL
        inp = input[b0:b0 + BL]
        # feats [BL, QTR, 64, 2, 128, S]: c-major then (h, p, s) to match xt cols
        f = inp[:, 12:76].astype(np.float16)                 # (BL,64,1024,32)
        f = f.reshape(BL, 64, QTR, 2, 128, S).transpose(0, 2, 1, 3, 4, 5)
        feats = np.ascontiguousarray(f)
        # geo [BL, 128, 3f, 3j, NQ, S] fp16 point-major
        g = inp[:, 3:12].astype(np.float16)                  # (BL,9,1024,32)
        g = g.reshape(BL, 3, 3, NQ, 128, S).transpose(0, 4, 1, 2, 3, 5)
        geo = np.ascontiguousarray(g)
        # normp[b, pt, xyz, q] = normal[b, q*128+pt, xyz]
        normp = np.ascontiguousarray(
            normal[b0:b0 + BL].reshape(BL, NQ, 128, 3).transpose(0, 2, 3, 1))
        in_maps.append({
            "feats": feats, "geo": geo, "normp": normp,
            "w1c": w1c, "w2T": w2T, "b1c": b1c, "b2c": b2c,
        })
    return in_maps


def assemble_output(results):
    outs = []
    for r in results:
        outp = r["outp"]                      # (BL,128,P)
        outa = r["outa"]                      # (BL,128,3,NQ)
        azi = outa.transpose(0, 2, 3, 1).reshape(BL, 3, P)
        outs.append(np.concatenate([azi, outp], axis=1))   # (BL,131,P)
    return np.concatenate(outs, axis=0)


def kernel(input, normal, w1, b1, w2, b2, _trace=False):
    nc = _get_program()
    in_maps = make_in_maps(input, normal, w1, b1, w2, b2)
    res = run_bass_kernel_spmd(nc, in_maps, core_ids=list(range(NCORES)), trace=_trace)
    out = assemble_output(res.results)
    if _trace:
        return out, res
    return out
